# revision 85
# baseline (speedup 1.0000x reference)
"""EnergyAE loss kernel for Trainium2 (Bass/Tile), 8-core data-parallel.

512-sample batch sharded 64/core; weights replicated. Returns the same
5-tuple as the reference: (neg_log_prob, recon_loss, latent_energy,
logdet_loss, sigma), each (512,) float32.

v3 design vs baseline (682us -> 315us):
  - dec_W2 loaded ONCE as fp8e4m3 (direct casting DMA on the gpsimd
    SWDGE queue, raw values sit in e4m3's normal range), resident in
    SBUF; dec1, the J loop, and the final recon all read it from SBUF
    (removes 48MB of HBM streaming)
  - enc_W1 full strips over SP/Act/gpsimd queues, biases folded into
    the end of each PSUM accumulation group
  - J / dec1 / recon matmuls in fp8 DoubleRow (0.5 cyc/row, paired
    128-contractions); Vaug carries a x16 scale so JTJ is 256x, undone
    in the sigma^-2 combine; JTJ/g in bf16 (f32r pays 4 cyc/row below
    256-wide)
  - W2^T blocks for g built with identity matmuls (fp8 transpose is
    not supported) and consumed by a DoubleRow pg over dc pairs
  - hess computed directly in per-sample row layout via K[h,(i,j)] =
    w1d[i,h]w1d[j,h]; the mask-matmul machinery of the baseline is gone
  - eigmin via Householder tridiagonalization + 2-round Sturm
    multisection; Cholesky + triangular inverse for logdet/trace/solve
"""
import numpy as np

import concourse.bass as bass
import concourse.tile as tile
from concourse import mybir

F32 = mybir.dt.float32
F32R = mybir.dt.float32r
BF16 = mybir.dt.bfloat16
FP8 = mybir.dt.float8e4
I32 = mybir.dt.int32
AX = mybir.AxisListType
ALU = mybir.AluOpType
ACTF = mybir.ActivationFunctionType
AP = bass.AP

D, H, N, BS = 3072, 2048, 16, 512
NCORES = 8
B = BS // NCORES            # 64
KC_H = H // 128             # 16
KC_D = D // 128             # 24
NGRP = B // 8               # 8
PACK = NGRP * 128           # 1024
BN = B * N                  # 1024
NSHIFT = 64                 # Sturm multisection grid
NSTURM = 2                  # multisection iterations


def _sap(t, offset, *dims):
    base = t[:]
    return AP(tensor=base.tensor, offset=base.offset + offset, ap=list(dims))


def split_excess_waits(nc, max_waits=1):
    """This walrus build accepts only one sync wait per instruction: move
    excess waits onto same-engine NoOps inserted just before."""
    n = 0
    for f in nc.m.functions:
        for bb in f.blocks:
            out = []
            for ins in bb.instructions:
                si = getattr(ins, "sync_info", None)
                ow = list(si.on_wait) if (si is not None and si.on_wait) else []
                if len(ow) > max_waits:
                    si.on_wait = ow[-max_waits:]
                    for w in ow[:-max_waits]:
                        n += 1
                        out.append(mybir.InstNoOp(
                            name=f"I-waitsplit-{n}",
                            sync_info=mybir.SyncInfo(on_wait=[w], on_update=[]),
                            bass_nofuse=True,
                            engine=ins.engine,
                        ))
                out.append(ins)
            bb.instructions = out
    return n


def build_module(debug=False):
    from contextlib import ExitStack

    nc = bass.Bass("TRN2", target_bir_lowering=False, debug=False,
                   num_devices=NCORES)

    x_d = nc.declare_dram_parameter("x", [B, D], F32R, isOutput=False)
    eps_d = nc.declare_dram_parameter("eps", [B, N], F32, isOutput=False)
    eW1_d = nc.declare_dram_parameter("enc_W1", [D, H], F32R, isOutput=False)
    eb1_d = nc.declare_dram_parameter("enc_b1", [H], F32R, isOutput=False)
    eWmu_d = nc.declare_dram_parameter("enc_Wmu", [H, N], F32R, isOutput=False)
    ebmu_d = nc.declare_dram_parameter("enc_bmu", [N], F32R, isOutput=False)
    eWls_d = nc.declare_dram_parameter("enc_Wls", [H, 1], F32R, isOutput=False)
    ebls_d = nc.declare_dram_parameter("enc_bls", [1], F32R, isOutput=False)
    dW1_d = nc.declare_dram_parameter("dec_W1", [N, H], F32R, isOutput=False)
    db1_d = nc.declare_dram_parameter("dec_b1", [H], F32, isOutput=False)
    dW2_d = nc.declare_dram_parameter("dec_W2", [H, D], F32R, isOutput=False)
    db2_d = nc.declare_dram_parameter("dec_b2", [D], F32R, isOutput=False)
    out_d = nc.declare_dram_parameter("out", [B, 5], F32, isOutput=True)

    dbg = {}
    if debug:
        for name, shape in [
            ("dbg_h", [B, H]), ("dbg_zsig", [B, N + 1]),
            ("dbg_jtj", [B, N * N]), ("dbg_g", [128, KC_H * B]),
            ("dbg_dec1", [B, D]), ("dbg_hess", [B, N * N]),
            ("dbg_prec", [B, N * N]), ("dbg_tri", [B, 2 * N]),
            ("dbg_eig", [B, 4]), ("dbg_chol", [B, N * N]),
            ("dbg_zoff", [B, N]),
        ]:
            dbg[name] = nc.declare_dram_parameter(name, shape, F32,
                                                  isOutput=True)

    ctx = ExitStack()
    with tile.TileContext(nc) as tc, ctx:
        from contextlib import ExitStack as _ES
        per = ctx.enter_context(tc.tile_pool(name="per", bufs=1))
        sm = ctx.enter_context(tc.tile_pool(name="sm", bufs=1))
        psctx = _ES()
        _pscur = [None]

        def psum_phase(name):
            nonlocal psctx
            psctx.close()
            psctx = _ES()
            _pscur[0] = psctx.enter_context(
                tc.tile_pool(name=name, bufs=1, space="PSUM"))
            return _pscur[0]
        V = nc.vector
        SC = nc.scalar
        PL = nc.gpsimd
        SY = nc.sync
        QS = [SY, SC, PL]

        def dbg_dump(name, src_ap, cast=False):
            if not debug:
                return
            nc.sync.dma_start(out=dbg[name][:],
                              in_=src_ap.bitcast(F32) if cast else src_ap)

        # ================= S0: patterns & small loads =================
        io_rowf = sm.tile([128, 128], F32)
        PL.iota(io_rowf[:], pattern=[[1, 128]], base=0,
                channel_multiplier=0, allow_small_or_imprecise_dtypes=True)
        pidx = sm.tile([128, 1], F32)
        PL.iota(pidx[:], pattern=[[0, 1]], base=0, channel_multiplier=1,
                allow_small_or_imprecise_dtypes=True)
        # eye-row mask for the per-sample row layout: [1,256], 1 where i==j
        eyei = sm.tile([1, N * N], F32, tag="dchx", bufs=2)
        PL.iota(eyei[:], pattern=[[1, N], [0, N]], base=0,
                channel_multiplier=0, allow_small_or_imprecise_dtypes=True)
        eyej = sm.tile([1, N * N], F32, tag="dchx", bufs=2)
        PL.iota(eyej[:], pattern=[[0, N], [1, N]], base=0,
                channel_multiplier=0, allow_small_or_imprecise_dtypes=True)
        eyerow = sm.tile([1, N * N], BF16)
        V.tensor_tensor(out=eyerow[:], in0=eyei[:], in1=eyej[:],
                        op=ALU.is_equal)
        ident = sm.tile([128, 128], F32R)
        V.tensor_scalar(out=ident[:], in0=io_rowf[:], scalar1=pidx[:],
                        scalar2=None, op0=ALU.is_equal)
        identB = sm.tile([128, 128], BF16)
        V.tensor_copy(identB[:], ident[:].bitcast(F32))
        ones_row = sm.tile([1, 128], F32R)
        V.tensor_scalar(out=ones_row[:], in0=io_rowf[0:1, :], scalar1=0.0,
                        scalar2=None, op0=ALU.is_ge)
        onesB = sm.tile([1, 128], BF16)
        V.tensor_copy(onesB[:], ones_row[:].bitcast(F32))
        ones64 = sm.tile([1, 128], F32R)
        V.tensor_scalar(out=ones64[:], in0=ones_row[:].bitcast(F32),
                        scalar1=64.0, scalar2=None, op0=ALU.mult)
        ident8 = sm.tile([128, 128], FP8)
        V.tensor_copy(ident8[:], ident[:].bitcast(F32))

        x_sb = per.tile([B, D], F32R, tag="sKx")
        SY.dma_start(out=x_sb, in_=x_d[:])
        eps_sb = per.tile([B, N], F32)
        SC.dma_start(out=eps_sb, in_=eps_d[:])
        db1c = sm.tile([128, KC_H], F32)
        SC.dma_start(out=db1c, in_=AP(tensor=db1_d, offset=0,
                                      ap=[[1, 128], [128, KC_H]]))
        muls = per.tile([128, KC_H, N + 1], BF16)
        PL.dma_start(out=muls[:, :, 0:N],
                     in_=AP(tensor=eWmu_d, offset=0,
                            ap=[[N, 128], [128 * N, KC_H], [1, N]])
                     .bitcast(F32))
        PL.dma_start(out=muls[:, :, N:N + 1],
                     in_=AP(tensor=eWls_d, offset=0,
                            ap=[[1, 128], [128, KC_H], [0, 1]]).bitcast(F32))
        bmur = sm.tile([1, N + 1], BF16)
        PL.dma_start(out=bmur[:, 0:N], in_=AP(tensor=ebmu_d, offset=0,
                                              ap=[[0, 1], [1, N]])
                     .bitcast(F32))
        PL.dma_start(out=bmur[:, N:N + 1],
                     in_=AP(tensor=ebls_d, offset=0, ap=[[0, 1], [1, 1]])
                     .bitcast(F32))


        psum_phase("ps0")

        def pe_transpose(dst_ap, src_ap, p, f, idt=None, dt_=F32R):
            pt = _pscur[0].tile([128, 128], dt_, name="pt_stage",
                                tag="pt_stage", bufs=2)
            nc.tensor.transpose(pt[:f, :p], src_ap,
                                (ident if idt is None else idt)[:p, :p])
            V.tensor_copy(dst_ap, pt[:f, :p])

        xT = per.tile([128, KC_D, B], F32R, tag="xT_dT")
        for dc in range(KC_D):
            pe_transpose(xT[:, dc, :], x_sb[:, dc * 128:(dc + 1) * 128], B, 128)
        # dec_W1 staged briefly (slot shared with hT/JTJsb), transposed to
        # w1dT, then the staging dies.
        w1d_stg = per.tile([N, H], F32R, name="w1d_stg", tag="hT")
        SY.dma_start(out=w1d_stg, in_=dW1_d[:])
        w1dT = per.tile([128, KC_H, N], F32R)
        for kc in range(KC_H):
            pe_transpose(w1dT[:, kc, :], w1d_stg[:, kc * 128:(kc + 1) * 128],
                         N, 128)

        # ================= resident W2 (fp8, x64 scaled) =================
        w2res = per.tile([128, KC_H, D], FP8, tag="w2res")

        # ================= S1: encoder h  +  W2 load =================
        ps = _pscur[0]
        ph = [ps.tile([B, 512], F32, name=f"ph{i}") for i in range(4)]
        # W2: 16 strips fp8-cast straight into the resident tile on the
        # gpsimd SWDGE queue; W2 values sit in e4m3's normal range unscaled.
        for kc in range(KC_H):
            PL.dma_start(out=w2res[:, kc, :],
                         in_=dW2_d[kc * 128:(kc + 1) * 128, :].bitcast(F32))
        # W1 full strips: 4 on the gpsimd SWDGE queue (after W2), 10 each on
        # SP/Act; bias is folded in at the end of the accumulation.
        for kc in range(KC_D):
            w1s = per.tile([128, H], F32R, name="w1s", tag="w1s",
                           bufs=4)
            q = PL if kc % 6 == 5 else (SY if kc % 2 == 0 else SC)
            q.dma_start(out=w1s, in_=eW1_d[kc * 128:(kc + 1) * 128, :])
            for nck in range(4):
                nc.tensor.matmul(ph[nck][:], xT[:, kc, :],
                                 w1s[:, nck * 512:(nck + 1) * 512],
                                 start=(kc == 0), stop=False,
                                 skip_group_check=(kc != 0))
        for nck in range(4):
            eb1c = sm.tile([1, 512], F32R, name="eb1c", tag="b512", bufs=1)
            SC.dma_start(out=eb1c, in_=AP(tensor=eb1_d, offset=nck * 512,
                                          ap=[[0, 1], [1, 512]]))
            nc.tensor.matmul(ph[nck][:], ones_row[:, 0:B], eb1c[:],
                             start=False, stop=True)
        # W2^T blocks (fp8 via identity matmul) precomputed while the PE is
        # otherwise idle during the load/feature phase
        w2t8 = per.tile([128, KC_D, KC_H, 128], FP8, tag="w2t8")
        wtp8 = w2t8[:].ap[0][0]
        for dc in range(KC_D):
            for jb in range(2):
                ptr = _pscur[0].tile([128, 8, 128], F32, name="ptr",
                                     tag="ptrst", bufs=1)
                for kk in range(8):
                    nc.tensor.matmul(ptr[:, kk, :],
                                     w2res[:, jb * 8 + kk,
                                           dc * 128:(dc + 1) * 128],
                                     ident8[:], start=True, stop=True)
                dst8 = _sap(w2t8, dc * KC_H * 128 + jb * 8 * 128,
                            [wtp8, 128], [128, 8], [1, 128])
                if dc % 3 != 2:
                    V.tensor_copy(dst8, ptr[:])
                else:
                    SC.copy(dst8, ptr[:])

        h_sb = per.tile([B, H], BF16, tag="hG")
        for nck in range(4):
            SC.activation(h_sb[:, nck * 512:(nck + 1) * 512], ph[nck][:],
                          ACTF.Tanh)
        hT = per.tile([128, KC_H, B], BF16, tag="hT")
        for kc in range(KC_H):
            pe_transpose(hT[:, kc, :], h_sb[:, kc * 128:(kc + 1) * 128], B,
                         128, idt=identB, dt_=BF16)

        # ================= S2: z_star / sigma =================
        ps = psum_phase("ps2")
        pz = ps.tile([N, B], F32, name="pz")
        nc.tensor.matmul(pz[:], bmur[:, 0:N], onesB[:, 0:B], start=True,
                         stop=False)
        for kc in range(KC_H):
            nc.tensor.matmul(pz[:], muls[:, kc, 0:N], hT[:, kc, :],
                             start=False, stop=(kc == KC_H - 1),
                             skip_group_check=(kc != KC_H - 1))
        pzs = ps.tile([1, B], F32, name="pzs")
        nc.tensor.matmul(pzs[:], bmur[:, N:N + 1], onesB[:, 0:B],
                         start=True, stop=False)
        for kc in range(KC_H):
            nc.tensor.matmul(pzs[:], muls[:, kc, N:N + 1], hT[:, kc, :],
                             start=False, stop=(kc == KC_H - 1),
                             skip_group_check=(kc != KC_H - 1))
        zT = per.tile([N, B], F32R)
        V.tensor_copy(zT[:], pz[:])
        sig_row = sm.tile([1, B], F32R)
        SC.activation(sig_row[:], pzs[:], ACTF.Exp)
        invsigT = sm.tile([1, B], F32R)
        with nc.allow_low_precision(reason="fp32r bits are full fp32 here"):
            V.reciprocal(invsigT[:], sig_row[:].bitcast(F32))
        pb = ps.tile([128, B], F32, name="pb")
        nc.tensor.matmul(pb[:], ones_row[:, 0:128], invsigT[:],
                         start=True, stop=True)
        invsig_bc = per.tile([128, B], F32)
        V.tensor_copy(invsig_bc[:], pb[:])
        # batch layout via matmul transposes: zsig (B, 17)
        pzb = ps.tile([B, N], F32, name="pzb")
        nc.tensor.matmul(pzb[:], zT[:], ident[0:N, 0:N],
                         start=True, stop=True)
        psb = ps.tile([B, 64], F32, name="psb")
        nc.tensor.matmul(psb[:], sig_row[:], ones_row[:, 0:64],
                         start=True, stop=True)
        zsig = per.tile([B, N + 1], F32R)
        V.tensor_copy(zsig[:, 0:N], pzb[:])
        V.tensor_copy(zsig[:, N:N + 1], psb[:, 0:1])
        z_b = zsig[:, 0:N].bitcast(F32)
        sig_b = zsig[:, N:N + 1].bitcast(F32)
        dbg_dump("dbg_zsig", zsig[:], cast=True)
        invsig_b = sm.tile([B, 1], F32)
        V.reciprocal(invsig_b[:], sig_b)
        invsig2_b = sm.tile([B, 1], F32)
        V.tensor_tensor(out=invsig2_b[:], in0=invsig_b[:], in1=invsig_b[:],
                        op=ALU.mult)
        invsig2_s = sm.tile([B, 1], F32)
        V.tensor_scalar(out=invsig2_s[:], in0=invsig2_b[:],
                        scalar1=float(2.0 ** -8), scalar2=None, op0=ALU.mult)

        # ================= S3: decoder features at z_star =================
        tT = per.tile([128, KC_H, B], FP8)
        sT = per.tile([128, KC_H, B], F32, tag="sKx")
        wT = per.tile([128, KC_H, B], F32, tag="wT")
        ps = psum_phase("ps3")
        for kc in range(KC_H):
            w1dc = sm.tile([N, 128], F32R, name="w1dc", tag="w1dc", bufs=2)
            pe_transpose(w1dc[:], w1dT[:, kc, :], 128, N)
            pa = ps.tile([128, B], F32, name="pa", tag="pa", bufs=2)
            nc.tensor.matmul(pa[:], w1dc[:], zT[:],
                             start=True, stop=True)
            SC.activation(tT[:, kc, :], pa[:], ACTF.Tanh,
                          bias=db1c[:, kc:kc + 1])
            tf = sm.tile([128, B], F32, name="tf", tag="tf", bufs=2)
            SC.activation(tf[:], pa[:], ACTF.Tanh, bias=db1c[:, kc:kc + 1])
            t2f = sm.tile([128, B], F32, name="t2f", tag="t2f", bufs=2)
            SC.activation(t2f[:], tf[:], ACTF.Square)
            V.tensor_scalar(out=sT[:, kc, :], in0=t2f[:], scalar1=-1.0,
                            scalar2=1.0, op0=ALU.mult, op1=ALU.add)
            PL.tensor_tensor(out=wT[:, kc, :], in0=tf[:], in1=sT[:, kc, :],
                             op=ALU.mult)
        Vaug = per.tile([128, KC_H, BN], FP8, tag="Vbig")
        vp = Vaug[:].ap[0][0]
        sp_ = sT[:].ap[0][0]
        wtp = w1dT[:].ap[0][0]
        for kc in range(KC_H):
            V.scalar_tensor_tensor(
                out=_sap(Vaug, kc * BN, [vp, 128], [N, B], [1, N]),
                in0=_sap(sT, kc * B, [sp_, 128], [1, B], [0, N]),
                scalar=16.0,
                in1=_sap(w1dT, kc * N, [wtp, 128], [0, B], [1, N]).bitcast(F32),
                op0=ALU.mult, op1=ALU.mult)

        # ================= S3.5: dec1 from resident W2 =================
        ps = psum_phase("ps35")
        pd = [ps.tile([B, 512], F32, name=f"pd{i}") for i in range(6)]
        DR0 = mybir.MatmulPerfMode.DoubleRow
        tp_ = tT[:].ap[0][0]
        wrp = w2res[:].ap[0][0]
        for kp in range(KC_H // 2):
            for nck in range(6):
                nc.tensor.matmul(
                    pd[nck][:],
                    _sap(tT, 2 * kp * B, [tp_, 128], [B, 2], [1, B]),
                    _sap(w2res, 2 * kp * D + nck * 512, [wrp, 128], [D, 2],
                         [1, 512]),
                    start=(kp == 0), stop=False,
                    skip_group_check=(kp != 0), perf_mode=DR0)
        for nck in range(6):
            b2s0 = sm.tile([1, 512], F32R, name="b2s0", tag="dchx", bufs=2)
            SC.dma_start(out=b2s0, in_=AP(tensor=db2_d, offset=nck * 512,
                                          ap=[[0, 1], [1, 512]]))
            nc.tensor.matmul(pd[nck][:], ones_row[:, 0:B], b2s0[:],
                             start=False, stop=True)
        # dT = (xT - dec1T)/sigma built in transposed layout: copy each dec1
        # PSUM chunk to bf16, transpose it, and combine with resident xT.
        dT_all = per.tile([128, KC_D, B], FP8, tag="b512")
        for nck in range(6):
            dch = sm.tile([B, 512], BF16, name="dch", tag="dchx", bufs=2)
            V.tensor_copy(dch[:], pd[nck][:])
            for j in range(4):
                dc = nck * 4 + j
                ptd = _pscur[0].tile([128, B], BF16, name="ptd", tag="ptd",
                                     bufs=2)
                nc.tensor.transpose(ptd[:, 0:B], dch[:, j * 128:(j + 1) * 128],
                                    identB[0:B, 0:B])
                ptf = sm.tile([128, B], F32, name="ptf", tag="tf", bufs=2)
                V.tensor_copy(ptf[:], ptd[:, 0:B])
                V.scalar_tensor_tensor(out=ptf[:], in0=ptf[:], scalar=-1.0,
                                       in1=xT[:, dc, :].bitcast(F32),
                                       op0=ALU.mult, op1=ALU.add)
                V.tensor_tensor(out=dT_all[:, dc, :], in0=ptf[:],
                                in1=invsig_bc[:], op=ALU.mult)

        # ================= S4: J / JTJ / g loop (from resident W2) ========
        ps = psum_phase("ps4")
        pJ = ps.tile([128, BN], F32, name="pJ")                # 2 banks
        pJTJ = ps.tile([128, NGRP, 128], F32, name="pJTJ")     # 2 banks
        pg = ps.tile([128, KC_H, B], F32, name="pgall")        # 2 banks
        DR = mybir.MatmulPerfMode.DoubleRow
        dtp8 = dT_all[:].ap[0][0]
        for dcp in range(KC_D // 2):
            for dci in range(2):
                dc = dcp * 2 + dci
                for lo, hi in ((0, 512), (512, 1024)):
                    for kp in range(KC_H // 2):
                        nc.tensor.matmul(
                            pJ[:, lo:hi],
                            _sap(w2res, 2 * kp * D + dc * 128,
                                 [w2res[:].ap[0][0], 128], [D, 2], [1, 128]),
                            _sap(Vaug, 2 * kp * BN + lo,
                                 [vp, 128], [BN, 2], [1, hi - lo]),
                            start=(kp == 0), stop=(kp == KC_H // 2 - 1),
                            skip_group_check=(kp not in (0, KC_H // 2 - 1)),
                            perf_mode=DR)
                Jsb = per.tile([128, BN], BF16, name="Jsb", tag="Jsb", bufs=1)
                if dc % 2 == 0:
                    V.tensor_copy(Jsb[:], pJ[:])
                else:
                    SC.copy(Jsb[:], pJ[:])
                for g in range(NGRP):
                    st = (dc == 0 and g in (0, 4))
                    sp2 = (dc == KC_D - 1 and g in (3, 7))
                    nc.tensor.matmul(pJTJ[:, g, :],
                                     Jsb[:, g * 128:(g + 1) * 128],
                                     Jsb[:, g * 128:(g + 1) * 128],
                                     start=st, stop=sp2,
                                     skip_group_check=not (st or sp2))
            for kc in range(KC_H):
                st = (dcp == 0 and kc in (0, 8))
                sp2 = (dcp == KC_D // 2 - 1 and kc in (7, 15))
                nc.tensor.matmul(
                    pg[:, kc, :],
                    _sap(w2t8, 2 * dcp * KC_H * 128 + kc * 128,
                         [wtp8, 128], [KC_H * 128, 2], [1, 128]),
                    _sap(dT_all, 2 * dcp * B, [dtp8, 128], [B, 2], [1, B]),
                    start=st, stop=sp2, skip_group_check=not (st or sp2),
                    perf_mode=DR)
            if dcp == 0:
                # K[h, i*16+j] = w1d[i,h]*w1d[j,h]  (input-independent)
                K = per.tile([128, KC_H, N * N], BF16, tag="sKx")
                kp_ = K[:].ap[0][0]
                for kc in range(KC_H):
                    PL.tensor_tensor(
                        out=_sap(K, kc * N * N, [kp_, 128], [N, N], [1, N]),
                        in0=_sap(w1dT, kc * N, [wtp, 128], [1, N], [0, N])
                        .bitcast(F32),
                        in1=_sap(w1dT, kc * N, [wtp, 128], [0, N], [1, N])
                        .bitcast(F32),
                        op=ALU.mult)

        # ================= S4b: hess in row layout =================
        gsb = per.tile([128, KC_H, B], F32, tag="hG")
        V.tensor_copy(gsb[:], pg[:])
        dbg_dump("dbg_g", gsb[:].rearrange("p a b -> p (a b)"))
        # c = 2*t*s*g/sigma: wT <- wT*g (f32, in place), then cT = wT/sigma
        V.tensor_tensor(out=wT[:].rearrange("p a b -> p (a b)"),
                        in0=wT[:].rearrange("p a b -> p (a b)"),
                        in1=gsb[:].rearrange("p a b -> p (a b)"),
                        op=ALU.mult)
        cT = per.tile([128, KC_H, B], BF16, tag="Jsb")
        cp2 = cT[:].ap[0][0]
        V.scalar_tensor_tensor(
            out=_sap(cT, 0, [cp2, 128], [B, KC_H], [1, B]),
            in0=_sap(wT, 0, [wT[:].ap[0][0], 128], [B, KC_H], [1, B]),
            scalar=2.0,
            in1=_sap(invsig_bc, 0, [invsig_bc[:].ap[0][0], 128], [0, KC_H],
                     [1, B]),
            op0=ALU.mult, op1=ALU.mult)
        ps = psum_phase("ps4b")
        pH = ps.tile([B, N * N], F32, name="pH")
        nc.tensor.matmul(pH[:], onesB[:, 0:B], eyerow[:],
                         start=True, stop=False)
        for kc in range(KC_H):
            nc.tensor.matmul(pH[:], cT[:, kc, :], K[:, kc, :],
                             start=False, stop=(kc == KC_H - 1),
                             skip_group_check=(kc != KC_H - 1))

        # ================= S4c/d: combine + unpack =================
        JTJsb = per.tile([128, PACK], F32, tag="hT")
        V.tensor_copy(JTJsb[:], pJTJ[:].rearrange("p a b -> p (a b)"))
        prec = per.tile([B, N * N], F32, tag="b512")
        ppj = JTJsb[:].ap[0][0]
        ppr = prec[:].ap[0][0]
        # sample b = g*8+r sits at rows r*16..r*16+16, cols g*128+r*16+j of
        # the packed tile; per-sample gather DMAs over the 3 queues.
        for b in range(B):
            g, rr_ = b // 8, b % 8
            QS[b % 3].dma_start(
                out=prec[b:b + 1, :],
                in_=_sap(JTJsb, rr_ * 16 * ppj + g * 128 + rr_ * 16,
                         [ppj, N], [1, N]))
        dbg_dump("dbg_jtj", prec[:])
        if debug:
            hrows = sm.tile([B, N * N], F32, name="hrows")
            V.tensor_copy(hrows[:], pH[:])
            nc.sync.dma_start(out=dbg["dbg_hess"][:], in_=hrows[:])
        V.scalar_tensor_tensor(out=prec[:], in0=prec[:],
                               scalar=invsig2_s[:], in1=pH[:],
                               op0=ALU.mult, op1=ALU.add)
        dbg_dump("dbg_prec", prec[:])

        # ================= S6: eigmin =================
        def pdiag(t, stride=N + 1, n=N, offset=0):
            return _sap(t, offset, [t[:].ap[0][0], B], [stride, n])

        absr = sm.tile([B, N], F32)
        V.tensor_reduce(out=absr[:],
                        in_=prec[:].rearrange("b (i j) -> b i j", i=N),
                        axis=AX.X, op=ALU.add, apply_absolute_value=True)
        dg = sm.tile([B, N], F32)
        V.tensor_copy(dg[:], pdiag(prec))
        absdg = sm.tile([B, N], F32)
        V.scalar_tensor_tensor(out=absdg[:], in0=dg[:], scalar=-1.0, in1=dg[:],
                               op0=ALU.mult, op1=ALU.max)
        lo_s = sm.tile([B, 1], F32)
        hi_s = sm.tile([B, 1], F32)
        lo_v = sm.tile([B, N], F32)
        V.tensor_tensor(out=lo_v[:], in0=dg[:], in1=absdg[:], op=ALU.add)
        V.tensor_tensor(out=lo_v[:], in0=lo_v[:], in1=absr[:], op=ALU.subtract)
        V.tensor_reduce(out=lo_s[:], in_=lo_v[:], axis=AX.X, op=ALU.min)
        V.tensor_reduce(out=hi_s[:], in_=dg[:], axis=AX.X, op=ALU.min)

        # --- Householder tridiagonalization ---
        A2 = per.tile([B, N * N], F32, tag="Vbig")
        V.tensor_copy(A2[:], prec[:])
        Ed = sm.tile([B, N], F32)
        V.memset(Ed[:], 0.0)
        ap2 = A2[:].ap[0][0]
        vvt = sm.tile([B, N], F32, name="vvt")
        vstep = vvt[:].ap[0][0]
        tmpm = sm.tile([B, N], F32, name="tmpm")
        qvt = sm.tile([B, N], F32, name="qvt")
        qstep = qvt[:].ap[0][0]
        omm = per.tile([B, N * N], F32, name="omm", tag="wT")
        omm2 = per.tile([B, N * N], F32, name="omm2", tag="tT")
        s1 = sm.tile([B, 1], F32, name="s1t")
        s2 = sm.tile([B, 1], F32, name="s2t")
        s3 = sm.tile([B, 1], F32, name="s3t")
        s4 = sm.tile([B, 1], F32, name="s4t")
        for k in range(N - 2):
            m = N - 1 - k
            xap = _sap(A2, (k + 1) * N + k, [ap2, B], [N, m])
            vt = vvt[:, 0:m]
            V.tensor_copy(vt, xap)
            V.tensor_tensor(out=tmpm[:, 0:m], in0=vt, in1=vt, op=ALU.mult)
            V.tensor_reduce(out=s1[:], in_=tmpm[:, 0:m], axis=AX.X, op=ALU.add)
            SC.activation(s2[:], s1[:], ACTF.Sqrt)
            V.tensor_scalar(out=s3[:], in0=vt[:, 0:1], scalar1=0.0,
                            scalar2=None, op0=ALU.is_ge)
            V.tensor_scalar(out=s3[:], in0=s3[:], scalar1=-2.0, scalar2=1.0,
                            op0=ALU.mult, op1=ALU.add)
            V.tensor_tensor(out=s3[:], in0=s3[:], in1=s2[:], op=ALU.mult)
            V.tensor_copy(Ed[:, k + 1:k + 2], s3[:])
            V.tensor_tensor(out=vt[:, 0:1], in0=vt[:, 0:1], in1=s3[:],
                            op=ALU.subtract)
            V.tensor_tensor(out=tmpm[:, 0:m], in0=vt, in1=vt, op=ALU.mult)
            V.tensor_reduce(out=s2[:], in_=tmpm[:, 0:m], axis=AX.X, op=ALU.add)
            V.tensor_scalar(out=s2[:], in0=s2[:], scalar1=1e-30, scalar2=None,
                            op0=ALU.max)
            V.reciprocal(s4[:], s2[:])
            V.tensor_scalar(out=s4[:], in0=s4[:], scalar1=2.0, scalar2=None,
                            op0=ALU.mult)
            asub = _sap(A2, (k + 1) * (N + 1), [ap2, B], [N, m], [1, m])
            V.tensor_tensor(
                out=omm[:, 0:m * m].rearrange("b (i j) -> b i j", i=m),
                in0=asub,
                in1=_sap(vvt, 0, [vstep, B], [0, m], [1, m]),
                op=ALU.mult)
            pvec = tmpm[:, 0:m]
            V.tensor_reduce(out=pvec,
                            in_=omm[:, 0:m * m].rearrange("b (i j) -> b i j",
                                                          i=m),
                            axis=AX.X, op=ALU.add)
            V.tensor_tensor(out=qvt[:, 0:m], in0=pvec, in1=vt, op=ALU.mult)
            V.tensor_reduce(out=s1[:], in_=qvt[:, 0:m], axis=AX.X, op=ALU.add)
            V.scalar_tensor_tensor(out=s1[:], in0=s1[:], scalar=0.5, in1=s4[:],
                                   op0=ALU.mult, op1=ALU.mult)
            V.tensor_scalar(out=qvt[:, 0:m], in0=vt, scalar1=s1[:],
                            scalar2=None, op0=ALU.mult)
            V.tensor_tensor(out=qvt[:, 0:m], in0=pvec, in1=qvt[:, 0:m],
                            op=ALU.subtract)
            V.tensor_tensor(
                out=omm[:, 0:m * m].rearrange("b (i j) -> b i j", i=m),
                in0=_sap(vvt, 0, [vstep, B], [1, m], [0, m]),
                in1=_sap(qvt, 0, [qstep, B], [0, m], [1, m]),
                op=ALU.mult)
            PL.tensor_tensor(
                out=omm2[:, 0:m * m].rearrange("b (i j) -> b i j", i=m),
                in0=_sap(qvt, 0, [qstep, B], [1, m], [0, m]),
                in1=_sap(vvt, 0, [vstep, B], [0, m], [1, m]),
                op=ALU.mult)
            V.tensor_tensor(
                out=omm[:, 0:m * m].rearrange("b (i j) -> b i j", i=m),
                in0=omm[:, 0:m * m].rearrange("b (i j) -> b i j", i=m),
                in1=omm2[:, 0:m * m].rearrange("b (i j) -> b i j", i=m),
                op=ALU.add)
            V.tensor_scalar(out=s4[:], in0=s4[:], scalar1=-1.0, scalar2=None,
                            op0=ALU.mult)
            V.scalar_tensor_tensor(
                out=asub,
                in0=omm[:, 0:m * m].rearrange("b (i j) -> b i j", i=m),
                scalar=s4[:], in1=asub, op0=ALU.mult, op1=ALU.add)
        Td = sm.tile([B, N], F32)
        V.tensor_copy(Td[:], pdiag(A2))
        nege2 = sm.tile([B, N], F32)
        V.tensor_tensor(out=nege2[:], in0=Ed[:], in1=Ed[:], op=ALU.mult)
        V.tensor_scalar(out=nege2[:], in0=nege2[:], scalar1=-1.0,
                        scalar2=-1e-30, op0=ALU.mult, op1=ALU.add)
        if debug:
            tri = sm.tile([B, 2 * N], F32, name="dbtri")
            V.tensor_copy(tri[:, 0:N], Td[:])
            V.tensor_copy(tri[:, N:2 * N], Ed[:])
            nc.sync.dma_start(out=dbg["dbg_tri"][:], in_=tri[:])

        # --- Sturm multisection ---
        iotaF = sm.tile([B, NSHIFT], F32)
        ioi2 = sm.tile([B, NSHIFT], I32)
        PL.iota(ioi2[:], pattern=[[1, NSHIFT]], base=1, channel_multiplier=0)
        V.tensor_copy(iotaF[:], ioi2[:])
        wid = sm.tile([B, 1], F32)
        V.tensor_tensor(out=wid[:], in0=hi_s[:], in1=lo_s[:], op=ALU.subtract)
        grid = sm.tile([B, NSHIFT], F32)
        dxm = per.tile([B, N, NSHIFT], F32, tag="hG")
        pp = sm.tile([B, NSHIFT], F32)
        rr = sm.tile([B, NSHIFT], F32)
        cnt = sm.tile([B, NSHIFT], F32)
        zz = sm.tile([B, NSHIFT], F32)
        stp = sm.tile([B, 1], F32)
        for it in range(NSTURM):
            V.tensor_scalar(out=stp[:], in0=wid[:],
                            scalar1=1.0 / (NSHIFT + 1.0), scalar2=None,
                            op0=ALU.mult)
            V.tensor_scalar(out=grid[:], in0=iotaF[:], scalar1=stp[:],
                            scalar2=lo_s[:], op0=ALU.mult, op1=ALU.add)
            V.tensor_tensor(out=dxm[:],
                            in0=_sap(Td, 0, [Td[:].ap[0][0], B], [1, N],
                                     [0, NSHIFT]),
                            in1=_sap(grid, 0, [grid[:].ap[0][0], B], [0, N],
                                     [1, NSHIFT]),
                            op=ALU.subtract)
            V.tensor_copy(pp[:], dxm[:, 0, :])
            V.tensor_scalar(out=cnt[:], in0=pp[:], scalar1=1e-25, scalar2=None,
                            op0=ALU.is_lt)
            for i in range(1, N):
                V.reciprocal(rr[:], pp[:])
                V.scalar_tensor_tensor(out=pp[:], in0=rr[:],
                                       scalar=nege2[:, i:i + 1],
                                       in1=dxm[:, i, :], op0=ALU.mult,
                                       op1=ALU.add)
                V.scalar_tensor_tensor(out=cnt[:], in0=pp[:], scalar=1e-25,
                                       in1=cnt[:], op0=ALU.is_lt, op1=ALU.add)
            V.tensor_scalar(out=zz[:], in0=cnt[:], scalar1=0.0, scalar2=None,
                            op0=ALU.is_equal)
            V.tensor_reduce(out=s1[:], in_=zz[:], axis=AX.X, op=ALU.add)
            V.scalar_tensor_tensor(out=lo_s[:], in0=s1[:], scalar=stp[:],
                                   in1=lo_s[:], op0=ALU.mult, op1=ALU.add)
            V.tensor_copy(wid[:], stp[:])
        eigmin = sm.tile([B, 1], F32)
        V.tensor_scalar(out=eigmin[:], in0=wid[:], scalar1=0.5,
                        scalar2=None, op0=ALU.mult)
        V.tensor_tensor(out=eigmin[:], in0=lo_s[:], in1=eigmin[:], op=ALU.add)
        delta = sm.tile([B, 1], F32)
        V.tensor_scalar(out=delta[:], in0=eigmin[:], scalar1=-1.0,
                        scalar2=10.0, op0=ALU.mult, op1=ALU.add)
        if debug:
            de = sm.tile([B, 4], F32, name="dbeig")
            V.tensor_copy(de[:, 0:1], eigmin[:])
            V.tensor_copy(de[:, 1:2], delta[:])
            V.tensor_copy(de[:, 2:3], lo_s[:])
            V.tensor_copy(de[:, 3:4], hi_s[:])
            nc.sync.dma_start(out=dbg["dbg_eig"][:], in_=de[:])

        # ================= S6b: Cholesky of Prec + delta*I =================
        U = A2  # reuse A2 storage: overwrite with a fresh copy of prec
        V.tensor_copy(U[:], prec[:])
        V.tensor_scalar(out=pdiag(U), in0=pdiag(U), scalar1=delta[:],
                        scalar2=None, op0=ALU.add)
        yks = sm.tile([B, N], F32)   # 1/sqrt(d_k) per step == 1/U[k,k]
        for k in range(N):
            m = N - 1 - k
            dkk = _sap(U, k * (N + 1), [ap2, B], [1, 1])
            V.reciprocal(s1[:], dkk)
            SC.activation(s2[:], s1[:], ACTF.Sqrt)       # ~1/sqrt(d)
            # Newton polish: y <- y*(1.5 - 0.5*d*y^2)
            V.tensor_tensor(out=s3[:], in0=s2[:], in1=s2[:], op=ALU.mult)
            V.tensor_scalar(out=s3[:], in0=s3[:], scalar1=dkk, scalar2=None,
                            op0=ALU.mult)
            V.tensor_scalar(out=s3[:], in0=s3[:], scalar1=-0.5, scalar2=1.5,
                            op0=ALU.mult, op1=ALU.add)
            V.tensor_tensor(out=s2[:], in0=s2[:], in1=s3[:], op=ALU.mult)
            V.tensor_copy(yks[:, k:k + 1], s2[:])
            rowap = _sap(U, k * (N + 1), [ap2, B], [1, m + 1])
            V.tensor_scalar(out=rowap, in0=rowap, scalar1=s2[:], scalar2=None,
                            op0=ALU.mult)
            if m > 0:
                urow = _sap(U, k * N + k + 1, [ap2, B], [1, m])
                V.tensor_copy(vvt[:, 0:m], urow)
                sub = _sap(U, (k + 1) * (N + 1), [ap2, B], [N, m], [1, m])
                V.tensor_tensor(
                    out=omm[:, 0:m * m].rearrange("b (i j) -> b i j", i=m),
                    in0=_sap(vvt, 0, [vstep, B], [1, m], [0, m]),
                    in1=_sap(vvt, 0, [vstep, B], [0, m], [1, m]),
                    op=ALU.mult)
                V.tensor_tensor(
                    out=sub,
                    in0=sub,
                    in1=omm[:, 0:m * m].rearrange("b (i j) -> b i j", i=m),
                    op=ALU.subtract)
        if debug:
            dbg_dump("dbg_chol", U[:])
        # logdet_loss = sum log U_kk
        udg = sm.tile([B, N], F32)
        V.tensor_copy(udg[:], pdiag(U))
        lud = sm.tile([B, N], F32)
        logdet = sm.tile([B, 1], F32)
        SC.activation(lud[:], udg[:], ACTF.Ln, accum_out=logdet[:])

        # ================= S6c: X = U^{-1} (XT[c,r] layout) ==============
        XT = per.tile([B, N * N], F32, tag="dS6")
        V.memset(XT[:], 0.0)
        xtp = XT[:].ap[0][0]
        negy = sm.tile([B, N], F32)
        V.tensor_scalar(out=negy[:], in0=yks[:], scalar1=-1.0, scalar2=None,
                        op0=ALU.mult)
        for k in range(N - 1, -1, -1):
            m = N - 1 - k
            if m > 0:
                # S_c = sum_{j>k} U[k,j] * XT[c, j]
                V.tensor_copy(vvt[:, 0:m],
                              _sap(U, k * N + k + 1, [ap2, B], [1, m]))
                V.tensor_tensor(
                    out=omm[:, 0:N * m].rearrange("b (c j) -> b c j", c=N),
                    in0=_sap(XT, k + 1, [xtp, B], [N, N], [1, m]),
                    in1=_sap(vvt, 0, [vstep, B], [0, N], [1, m]),
                    op=ALU.mult)
                V.tensor_reduce(
                    out=tmpm[:, 0:N],
                    in_=omm[:, 0:N * m].rearrange("b (c j) -> b c j", c=N),
                    axis=AX.X, op=ALU.add)
                V.tensor_scalar(out=_sap(XT, k, [xtp, B], [N, N]),
                                in0=tmpm[:, 0:N], scalar1=negy[:, k:k + 1],
                                scalar2=None, op0=ALU.mult)
            V.tensor_tensor(out=_sap(XT, k * N + k, [xtp, B], [1, 1]),
                            in0=_sap(XT, k * N + k, [xtp, B], [1, 1]),
                            in1=yks[:, k:k + 1], op=ALU.add)
        # trinv = sum X^2 ; z_off = X @ eps  (scratch reuses omm/omm2)
        trinv = sm.tile([B, 1], F32)
        SC.activation(omm[:], XT[:], ACTF.Square, accum_out=trinv[:])
        zoffm = omm2[:].rearrange("b (i j) -> b i j", i=N)
        V.tensor_tensor(out=zoffm,
                        in0=_sap(XT, 0, [xtp, B], [1, N], [N, N]),
                        in1=_sap(eps_sb, 0, [eps_sb[:].ap[0][0], B], [0, N],
                                 [1, N]),
                        op=ALU.mult)
        z_off = sm.tile([B, N], F32)
        V.tensor_reduce(out=z_off[:], in_=zoffm, axis=AX.X, op=ALU.add)
        dbg_dump("dbg_zoff", z_off[:])
        z_samp = per.tile([B, N], F32R)
        V.tensor_tensor(out=z_samp[:], in0=z_b, in1=z_off[:], op=ALU.add)

        # latent_energy = 0.5*(|z*|^2 + trinv)
        zsq = sm.tile([B, N], F32, name="zsq")
        zn = sm.tile([B, 1], F32)
        SC.activation(zsq[:], z_b, ACTF.Square, accum_out=zn[:])
        lat = sm.tile([B, 1], F32)
        V.tensor_tensor(out=lat[:], in0=zn[:], in1=trinv[:], op=ALU.add)
        V.tensor_scalar(out=lat[:], in0=lat[:], scalar1=0.5, scalar2=None,
                        op0=ALU.mult)

        # ================= S5: recon at z_sample =================
        ps = psum_phase("ps5")
        zsT = per.tile([N, B], F32R)
        pe_transpose(zsT[:], z_samp[:], B, N)
        t2T = per.tile([128, KC_H, B], FP8, tag="hT")
        for kc in range(KC_H):
            w1dc2 = sm.tile([N, 128], F32R, name="w1dc2", tag="w1dc", bufs=2)
            pe_transpose(w1dc2[:], w1dT[:, kc, :], 128, N)
            pa2 = ps.tile([128, B], F32, name="pa2", tag="pa2", bufs=2)
            nc.tensor.matmul(pa2[:], w1dc2[:], zsT[:], start=True, stop=True)
            SC.activation(t2T[:, kc, :], pa2[:], ACTF.Tanh,
                          bias=db1c[:, kc:kc + 1])
        ps = psum_phase("ps5b")
        pr = [ps.tile([B, 512], F32, name=f"pr{i}") for i in range(6)]
        for nck in range(6):
            b2s = sm.tile([1, 512], F32R, name="b2s", tag="dchx", bufs=2)
            SC.dma_start(out=b2s, in_=AP(tensor=db2_d, offset=nck * 512,
                                         ap=[[0, 1], [1, 512]]))
            nc.tensor.matmul(pr[nck][:], ones_row[:, 0:B], b2s[:],
                             start=True, stop=False)
        t2p = t2T[:].ap[0][0]
        for kp in range(KC_H // 2):
            for nck in range(6):
                nc.tensor.matmul(
                    pr[nck][:],
                    _sap(t2T, 2 * kp * B, [t2p, 128], [B, 2], [1, B]),
                    _sap(w2res, 2 * kp * D + nck * 512, [wrp, 128], [D, 2],
                         [1, 512]),
                    start=False, stop=(kp == KC_H // 2 - 1),
                    skip_group_check=(kp != KC_H // 2 - 1),
                    perf_mode=mybir.MatmulPerfMode.DoubleRow)
        r2 = sm.tile([B, 1], F32)
        V.memset(r2[:], 0.0)
        for nck in range(6):
            xch = sm.tile([B, 512], F32, name="xch", tag="dchx", bufs=2)
            SY.dma_start(out=xch,
                         in_=AP(tensor=x_d, offset=nck * 512,
                                ap=[[D, B], [1, 512]]).bitcast(F32))
            rch = sm.tile([B, 512], F32, name="rch", tag="dchx", bufs=2)
            V.tensor_tensor(out=rch[:], in0=pr[nck][:], in1=xch[:],
                            op=ALU.subtract)
            racc = sm.tile([B, 1], F32, name="racc", tag="racc", bufs=2)
            SC.activation(rch[:], rch[:], ACTF.Square, accum_out=racc[:])
            V.tensor_tensor(out=r2[:], in0=r2[:], in1=racc[:], op=ALU.add)
        recon = sm.tile([B, 1], F32)
        V.scalar_tensor_tensor(out=recon[:], in0=r2[:], scalar=0.5,
                               in1=invsig2_b[:], op0=ALU.mult, op1=ALU.mult)

        # ================= outputs =================
        lsig = sm.tile([B, 1], F32)
        SC.activation(lsig[:], sig_b, ACTF.Ln)
        nlp = sm.tile([B, 1], F32)
        V.tensor_tensor(out=nlp[:], in0=recon[:], in1=lat[:], op=ALU.add)
        V.tensor_tensor(out=nlp[:], in0=nlp[:], in1=logdet[:], op=ALU.add)
        V.tensor_scalar(out=s1[:], in0=lsig[:], scalar1=float(D), scalar2=None,
                        op0=ALU.mult)
        V.tensor_tensor(out=nlp[:], in0=nlp[:], in1=s1[:], op=ALU.add)
        V.tensor_scalar(out=nlp[:], in0=nlp[:], scalar1=1.0 / D, scalar2=None,
                        op0=ALU.mult)
        outt = sm.tile([B, 5], F32)
        V.tensor_copy(outt[:, 0:1], nlp[:])
        V.tensor_copy(outt[:, 1:2], recon[:])
        V.tensor_copy(outt[:, 2:3], lat[:])
        V.tensor_copy(outt[:, 3:4], logdet[:])
        V.tensor_copy(outt[:, 4:5], sig_b)
        nc.sync.dma_start(out=out_d[:], in_=outt[:])
        psctx.close()

    return nc, dbg


MAX_LATENT_VAR = 0.1
_CACHE = {}


def _get_module(debug=False):
    key = bool(debug)
    if key not in _CACHE:
        nc, _ = build_module(debug)
        split_excess_waits(nc)
        # the CoreSim race detector mishandles gpsimd SWDGE DMAs (asserts on
        # missing fake sem updates); Tile's inserted deps provide ordering
        nc.detect_race_conditions = False
        _CACHE[key] = nc
    return _CACHE[key]


def kernel(**inputs):
    from concourse.bass_utils import run_bass_kernel_spmd
    nc = _get_module(False)
    x = np.asarray(inputs["x"], dtype=np.float32)
    eps = np.asarray(inputs["eps"], dtype=np.float32)
    rep = {k: np.asarray(v, dtype=np.float32) for k, v in inputs.items()
           if k not in ("x", "eps")}
    in_maps = []
    for c in range(NCORES):
        m = dict(rep)
        m["x"] = np.ascontiguousarray(x[c * B:(c + 1) * B])
        m["eps"] = np.ascontiguousarray(eps[0, c * B:(c + 1) * B, :])
        in_maps.append(m)
    r = run_bass_kernel_spmd(nc, in_maps, list(range(NCORES)))
    outs = np.concatenate([r.results[c]["out"] for c in range(NCORES)], axis=0)
    return (outs[:, 0], outs[:, 1], outs[:, 2], outs[:, 3], outs[:, 4])
